# revision 4
# baseline (speedup 1.0000x reference)
"""LocalGNN (2x GraphFilter[K=4] + ReLU + per-node readout) on 8 TRN2 cores.

Strategy: shard S column-wise (output-node dim) across the 8 cores; every
tap T_{k+1} = T_k @ S becomes, per core, psum[q, n'] += st_mb.T @ S_tile
with S streamed as the f32r moving operand (1 cyc/row) and st_mb the
[128, q] stationary built on-chip by PE-transposing the allgathered
previous tap. Taps chain through AllGather (rank-order concat == node
order). Filter-tap combinations / readout are folded into host-built
block-diagonal matrices so they are plain matmuls; bias+ReLU ride the
scalar engine.

kernel(**inputs) takes the FULL inputs from reference.setup_inputs() and
returns the FULL [8, 8, 16384] output.
"""

import os
import numpy as np

import concourse.bass as bass
import concourse.mybir as mybir
import concourse.tile as tile
from concourse import bacc
from concourse import bass_utils
from concourse.masks import make_identity

N = 16384
NC = 8          # cores
NL = N // NC    # 2048 local columns
B = 8
F1 = 16
F2 = 16
R = 8
KT = 4          # filter taps per layer
QP1 = 32        # layer-1 row dim (B=8 padded to 32 for partition alignment)
QP2 = B * F1    # 128
NMB = N // 128  # 128 m-blocks
SG = 2          # m-blocks per S DMA

ACT_F32R = True  # scalar-engine activation writes float32r directly


def _build():
    f32 = mybir.dt.float32
    f32r = mybir.dt.float32r
    nc = bacc.Bacc(
        "TRN2",
        target_bir_lowering=False,
        debug=False,
        enable_asserts=True,
        num_devices=NC,
    )
    S_d = nc.dram_tensor("S", [N, NL], f32r, kind="ExternalInput")
    xg_d = nc.dram_tensor("xg", [NC * QP1, NL], f32, kind="ExternalInput")
    xloc_d = nc.dram_tensor("xloc", [QP1, NL], f32r, kind="ExternalInput")
    C1_d = nc.dram_tensor("C1", [QP1, KT, 128], f32r, kind="ExternalInput")
    C2_d = nc.dram_tensor("C2", [128, KT, 128], f32r, kind="ExternalInput")
    CRO_d = nc.dram_tensor("CRO", [128, 64], f32r, kind="ExternalInput")
    b1_d = nc.dram_tensor("bias1", [128, 1], f32, kind="ExternalInput")
    b2_d = nc.dram_tensor("bias2", [128, 1], f32, kind="ExternalInput")
    bro_d = nc.dram_tensor("biasro", [64, 1], f32, kind="ExternalInput")
    out_d = nc.dram_tensor("out", [64, NL], f32, kind="ExternalOutput")

    rg = [list(range(NC))]

    with tile.TileContext(nc) as tc:
        with (
            tc.tile_pool(name="sconst", bufs=1) as sconst,
            tc.tile_pool(name="spool", bufs=2) as spool,
            tc.tile_pool(name="gpool", bufs=6) as gpool,
            tc.tile_pool(name="g1pool", bufs=2) as g1pool,
            tc.tile_pool(name="stpool", bufs=8) as stpool,
            tc.tile_pool(name="keep", bufs=1) as keep,
            tc.tile_pool(name="psacc", bufs=1, space="PSUM") as psacc,
            tc.tile_pool(name="pst", bufs=2, space="PSUM") as pst,
            tc.tile_pool(name="dpool", bufs=2, space="DRAM") as dpool,
        ):
            ident = sconst.tile([128, 128], f32, name="ident")
            make_identity(nc, ident[:])
            c1_sb = sconst.tile([QP1, KT, 128], f32r, name="c1_sb")
            nc.sync.dma_start(c1_sb[:], C1_d[:])
            c2_sb = sconst.tile([128, KT, 128], f32r, name="c2_sb")
            nc.sync.dma_start(c2_sb[:], C2_d[:])
            cro_sb = sconst.tile([128, 64], f32r, name="cro_sb")
            nc.sync.dma_start(cro_sb[:], CRO_d[:])
            b1_sb = sconst.tile([128, 1], f32, name="b1_sb")
            nc.sync.dma_start(b1_sb[:], b1_d[:])
            b2_sb = sconst.tile([128, 1], f32, name="b2_sb")
            nc.sync.dma_start(b2_sb[:], b2_d[:])
            bro_sb = sconst.tile([64, 1], f32, name="bro_sb")
            nc.sync.dma_start(bro_sb[:], bro_d[:])
            xloc_sb = sconst.tile([QP1, NL], f32r, name="xloc_sb")
            nc.sync.dma_start(xloc_sb[:], xloc_d[:])

            def load_g(src_rows, qp, pool):
                """src_rows: DRAM AP [NC*qp, NL] (gathered blocks) ->
                list of [128, NL] f32 SBUF tiles (natural row-blocks)."""
                ntiles = NC * qp // 128
                tiles = []
                for t in range(ntiles):
                    gt = pool.tile([128, NL], f32, name="gt", tag=f"g{qp}")
                    nc.sync.dma_start(gt[:], src_rows[t * 128:(t + 1) * 128, :])
                    tiles.append(gt)
                return tiles

            def tap(g_tiles, qp):
                """One graph-filter tap: acc[q, n'] = sum_m T[q, m] S[m, n0+n']."""
                acc = psacc.tile([qp, NL], f32, name="acc", tag="acc")
                per_tile = 128 // qp  # r-blocks per 128-row G tile
                for i in range(NMB // SG):
                    s_t = spool.tile([128, SG, NL], f32r, name="s_t", tag="S")
                    nc.sync.dma_start(
                        s_t[:],
                        S_d[i * SG * 128:(i + 1) * SG * 128, :].rearrange(
                            "(g p) j -> p g j", p=128
                        ),
                    )
                    for g in range(SG):
                        mb = SG * i + g
                        r, c = mb // 16, mb % 16
                        gt = g_tiles[r // per_tile]
                        pb = qp * (r % per_tile)
                        tp = pst.tile([128, qp], f32, name="tp", tag="tp")
                        nc.tensor.transpose(
                            tp[:],
                            gt[pb:pb + qp, c * 128:(c + 1) * 128],
                            ident[pb:pb + qp, pb:pb + qp],
                            tile_position=(pb, 0),
                        )
                        st = stpool.tile([128, qp], f32r, name="st", tag=f"st{qp}")
                        nc.vector.tensor_copy(st[:], tp[:])
                        for sp in range(4):
                            nc.tensor.matmul(
                                acc[:, sp * 512:(sp + 1) * 512],
                                st[:],
                                s_t[:, g, sp * 512:(sp + 1) * 512],
                                start=(mb == 0),
                                stop=(mb == NMB - 1),
                            )
                return acc

            def allgather(src_r, qp):
                """src_r: [qp, NL] f32r sbuf tile -> gathered DRAM [NC*qp, NL] f32."""
                f32v = src_r[:].bitcast(mybir.dt.float32)
                agi = dpool.tile([qp, NL], f32, name="agi", tag=f"agi{qp}")
                nc.sync.dma_start(agi[:], f32v)
                ago = dpool.tile(
                    [NC * qp, NL], f32, addr_space="Shared", name="ago",
                    tag=f"ago{qp}",
                )
                nc.gpsimd.collective_compute(
                    "AllGather",
                    mybir.AluOpType.bypass,
                    replica_groups=rg,
                    ins=[agi[:].opt()],
                    outs=[ago[:].opt()],
                )
                return ago

            # ---------------- Layer 1 ----------------
            g_cur = load_g(xg_d[:], QP1, g1pool)
            z_r = []
            for t in range(1, KT):
                acc = tap(g_cur, QP1)
                zr = keep.tile([QP1, NL], f32r, name="zr", tag=f"z{t}r")
                nc.vector.tensor_copy(zr[:], acc[:])
                z_r.append(zr)
                if t < KT - 1:
                    ago = allgather(zr, QP1)
                    g_cur = load_g(ago[:], QP1, g1pool)

            c1acc = psacc.tile([128, NL], f32, name="c1acc", tag="acc")
            rhs1 = [xloc_sb] + z_r
            for k in range(KT):
                for sp in range(4):
                    nc.tensor.matmul(
                        c1acc[:, sp * 512:(sp + 1) * 512],
                        c1_sb[:, k, :],
                        rhs1[k][:, sp * 512:(sp + 1) * 512],
                        start=(k == 0),
                        stop=(k == KT - 1),
                    )
            y1r = keep.tile([128, NL], f32r if ACT_F32R else f32,
                            name="y1r", tag="y1r")
            nc.scalar.activation(
                y1r[:], c1acc[:], mybir.ActivationFunctionType.Relu,
                bias=b1_sb[:],
            )

            # ---------------- Layer 2 ----------------
            ago = allgather(y1r, QP2)
            g_cur = load_g(ago[:], QP2, gpool)
            w_r = [y1r]
            for t in range(1, KT):
                acc = tap(g_cur, QP2)
                wr = keep.tile([QP2, NL], f32r, name="wr", tag=f"w{t}r")
                nc.vector.tensor_copy(wr[:], acc[:])
                w_r.append(wr)
                if t < KT - 1:
                    ago = allgather(wr, QP2)
                    g_cur = load_g(ago[:], QP2, gpool)

            c2acc = psacc.tile([128, NL], f32, name="c2acc", tag="acc")
            for k in range(KT):
                for sp in range(4):
                    nc.tensor.matmul(
                        c2acc[:, sp * 512:(sp + 1) * 512],
                        c2_sb[:, k, :],
                        w_r[k][:, sp * 512:(sp + 1) * 512],
                        start=(k == 0),
                        stop=(k == KT - 1),
                    )
            y2r = keep.tile([128, NL], f32r if ACT_F32R else f32,
                            name="y2r", tag="y2r")
            nc.scalar.activation(
                y2r[:], c2acc[:], mybir.ActivationFunctionType.Relu,
                bias=b2_sb[:],
            )

            # ---------------- Readout ----------------
            roacc = psacc.tile([64, NL], f32, name="roacc", tag="acc")
            for sp in range(4):
                nc.tensor.matmul(
                    roacc[:, sp * 512:(sp + 1) * 512],
                    cro_sb[:],
                    y2r[:, sp * 512:(sp + 1) * 512],
                    start=True,
                    stop=True,
                )
            outsb = keep.tile([64, NL], f32, name="outsb", tag="outsb")
            nc.scalar.activation(
                outsb[:], roacc[:], mybir.ActivationFunctionType.Identity,
                bias=bro_sb[:],
            )
            nc.sync.dma_start(out_d[:], outsb[:])

    nc.compile()
    return nc


_NC_CACHE = {}


def _prep_inputs(x, S, H1, b1, H2, b2, Wro, bro):
    x = np.asarray(x, np.float32)
    S = np.ascontiguousarray(np.asarray(S, np.float32))
    H1 = np.asarray(H1, np.float32)
    b1 = np.asarray(b1, np.float32)
    H2 = np.asarray(H2, np.float32)
    b2 = np.asarray(b2, np.float32)
    Wro = np.asarray(Wro, np.float32)
    bro = np.asarray(bro, np.float32)

    xbn = x[:, 0, :]                                    # [B, N]
    xg = np.zeros((NC, QP1, NL), np.float32)
    xg[:, :B, :] = xbn.reshape(B, NC, NL).transpose(1, 0, 2)
    xg = np.ascontiguousarray(xg.reshape(NC * QP1, NL))

    C1 = np.zeros((QP1, KT, 128), np.float32)
    for b in range(B):
        C1[b, :, b * F1:(b + 1) * F1] = H1[:, :, 0].T   # [k, g]
    C2 = np.zeros((128, KT, 128), np.float32)
    for b in range(B):
        C2[b * F1:(b + 1) * F1, :, b * F1:(b + 1) * F1] = H2.transpose(2, 1, 0)
    CRO = np.zeros((128, 64), np.float32)
    for b in range(B):
        CRO[b * F1:(b + 1) * F1, b * R:(b + 1) * R] = Wro.T

    bias1 = np.tile(b1[:, 0], B).reshape(128, 1).astype(np.float32)
    bias2 = np.tile(b2[:, 0], B).reshape(128, 1).astype(np.float32)
    biasro = np.tile(bro, B).reshape(64, 1).astype(np.float32)

    in_maps = []
    for r in range(NC):
        xloc = np.zeros((QP1, NL), np.float32)
        xloc[:B] = xbn[:, r * NL:(r + 1) * NL]
        in_maps.append({
            "S": np.ascontiguousarray(S[:, r * NL:(r + 1) * NL]),
            "xg": xg,
            "xloc": xloc,
            "C1": C1,
            "C2": C2,
            "CRO": CRO,
            "bias1": bias1,
            "bias2": bias2,
            "biasro": biasro,
        })
    return in_maps


def kernel(x, S, H1, b1, H2, b2, Wro, bro):
    if "nc" not in _NC_CACHE:
        _NC_CACHE["nc"] = _build()
    nc = _NC_CACHE["nc"]
    in_maps = _prep_inputs(x, S, H1, b1, H2, b2, Wro, bro)
    trace = bool(int(os.environ.get("GNN_TRACE", "0")))
    res = bass_utils.run_bass_kernel_spmd(
        nc, in_maps, core_ids=list(range(NC)), trace=trace,
    )
    kernel.last_results = res
    out = np.empty((B, R, N), np.float32)
    for r in range(NC):
        out[:, :, r * NL:(r + 1) * NL] = res.results[r]["out"].reshape(B, R, NL)
    return out
